# revision 1
# baseline (speedup 1.0000x reference)
"""DiagonalLSTM Trainium2 kernel.

Data-parallel over batch across 8 NeuronCores (16 samples/core). Per core, the
63-step diagonal scan runs with:
  - gates as 4x [128out, 32s*16b] matmuls per step, accumulated in PSUM:
    an itos tap (bf16, independent of the carry -- emitted first so the PE
    fills the recurrence-chain latency with it) and two stos taps (fp32r,
    1 cycle/column for N>=256);
  - sigmoid/tanh on ScalarE reading PSUM directly with the per-channel bias
    fused into the activation;
  - h_n = c_{n-1}*sigmoid(o) on VectorE, written straight into one persistent
    [128, 64col, 32s, 16b] SBUF history tile whose column n is h after step n
    (column 63 = h0), so the two stos taps of step n+1 are just two
    overlapping contiguous APs of column n -- no h copies, no double buffer;
  - c updates on VectorE, u = sig(i)*tanh(g) on GpSimd.
The free-dim layout is (s outer, b inner) so every matmul operand collapses to
one contiguous run. The skewed image is pre-gathered host-side into per-step
contiguous blocks (bf16) and only the rows a step actually needs are DMA'd.
Rows right of the valid band are skipped everywhere (the matmul band is padded
to >=16 rows to keep fp32r at full rate; the padded rows' garbage never flows
into read regions). Output rows are extracted from the history tile by VectorE
into contiguous staging tiles (4 rows each) and DMA'd with fully-contiguous
destination layout; the host reassembles (b,h,s,j) afterwards.

Cost-model timeline: ~160us/core; measured on hardware via an in-kernel
For_i repetition loop: ~190us/core per scan.
"""

import numpy as np

B, C, H, S = 128, 128, 128, 32
NCORES = 8
BL = B // NCORES
NSEQ = 2 * S - 1  # 63

_cache = {}


def _split_multi_waits(nc):
    # This walrus build accepts a single sync-wait per instruction; hoist
    # extras onto standalone EventSemaphore instructions on the same engine.
    import bass_rust
    import concourse.mybir as mybir

    n_split = 0
    for f in nc.m.functions:
        for bb in f.blocks:
            insts = bb.instructions
            new = []
            for ins in insts:
                si = ins.sync_info
                if si is not None and len(si.on_wait) > 1:
                    waits = list(si.on_wait)
                    for k, w in enumerate(waits[:-1]):
                        ev = mybir.InstEventSemaphore(
                            name=f"{ins.name}_splitw{k}",
                            engine=ins.engine,
                            ins=[],
                            outs=[],
                            sync_info=bass_rust.SyncInfo(on_wait=[w], on_update=[]),
                        )
                        new.append(ev)
                        n_split += 1
                    ins.sync_info = bass_rust.SyncInfo(
                        on_wait=[waits[-1]], on_update=list(si.on_update)
                    )
                new.append(ins)
            insts[:] = new
    return n_split


def _build(reps=1, stos_dt='f32r', skip_out=False):
    import concourse.bass as bass
    import concourse.mybir as mybir
    from concourse.tile import TileContext

    f32 = mybir.dt.float32
    f32r = mybir.dt.float32r
    bf16 = mybir.dt.bfloat16
    AF = mybir.ActivationFunctionType

    sdt = {"f32r": f32r, "bf16": bf16}[stos_dt]
    nc = bass.Bass()
    img = nc.dram_tensor("img", [C, NSEQ, S, BL], bf16, kind="ExternalInput")
    wst = nc.dram_tensor("wst", [8, H, H], sdt, kind="ExternalInput")
    wit = nc.dram_tensor("wit", [C, 4 * H], bf16, kind="ExternalInput")
    tbd = nc.dram_tensor("tbd", [H, 4], f32, kind="ExternalInput")
    h0b = nc.dram_tensor("h0b", [H, S, BL], sdt, kind="ExternalInput")
    c0b = nc.dram_tensor("c0b", [H, S, BL], f32, kind="ExternalInput")
    out = nc.dram_tensor("out", [S, H, BL, S], f32, kind="ExternalOutput")
    out_v = out.rearrange("s h b j -> h s b j")

    with TileContext(nc) as tc:
        with (
            tc.tile_pool(name="const", bufs=1) as cpool,
            tc.tile_pool(name="histp", bufs=1) as hpool,
            tc.tile_pool(name="state", bufs=3) as spool,
            tc.tile_pool(name="work", bufs=3) as wpool,
            tc.tile_pool(name="imgp", bufs=6) as ipool,
            tc.tile_pool(name="stgp", bufs=3) as gpool,
            tc.tile_pool(name="ps", bufs=2, space="PSUM") as ps,
        ):
            # weights: [tap*4+gate] -> lhsT [in 128, out 128], one DMA
            wstt = cpool.tile([H, 8, H], sdt, name="wstt")
            nc.gpsimd.dma_start(out=wstt, in_=wst.rearrange("k i o -> i k o"))
            wtiles = [wstt[:, k, :] for k in range(8)]
            wi = cpool.tile([C, 4 * H], bf16, name="wi")
            nc.gpsimd.dma_start(out=wi, in_=wit[:, :])
            tbt = cpool.tile([H, 4], f32, name="tbt")
            nc.gpsimd.dma_start(out=tbt, in_=tbd[:, :])

            # h history: col n = h after step n; col 63 = h_init
            hist = hpool.tile([H, 64, S, BL], sdt, name="hist")
            hist_r = hist.rearrange("p n s b -> p b n s")
            nc.sync.dma_start(out=hist[:, 63, :, :], in_=h0b[:, :, :])
            GATES = ((3, AF.Sigmoid), (0, AF.Sigmoid), (1, AF.Tanh), (2, AF.Sigmoid))

            import contextlib
            loop_ctx = tc.For_i(0, reps, 1) if reps > 1 else contextlib.nullcontext()
            with loop_ctx:
              c_prev = spool.tile([H, S, BL], f32, name="c_st")
              nc.sync.dma_start(out=c_prev, in_=c0b[:, :, :])
              for n in range(NSEQ):
                  s_lo = max(0, n - 31)     # true band start (elementwise ops)
                  sE = min(s_lo, 16)        # padded matmul band (keeps N >= 256)
                  s0E = max(sE, 1)          # rows receiving the shifted tap
                  cp = n - 1 if n > 0 else 63

                  # img DMA, two steps per transfer: for n<31 include the
                  # zero rows (they give the psum band its start=True
                  # coverage); for n>=31 rows [sE..s_lo) of the psum are
                  # never read, so their rhs rows can stay garbage.
                  if n % 2 == 0:
                      dlo = 0 if n < 31 else s_lo
                      n_hi = min(n + 2, NSEQ)
                      it2 = ipool.tile([C, 2, S, BL], bf16, name="imgt")
                      nc.sync.dma_start(
                          out=it2[:, : n_hi - n, dlo:, :],
                          in_=img[:, n:n_hi, dlo:, :],
                      )
                  it = it2[:, n % 2]

                  # Matmul order: o-gate group completes first so sigmoid(o)
                  # -> h -> next step's matmuls (the recurrence chain) starts as
                  # early as possible; the other gates' itos taps (independent
                  # of h_{n-1}) keep the PE busy while the chain drains.
                  ilo = sE if n < 31 else s_lo  # itos band start
                  pt = []
                  PBUFS = {3: 3, 0: 2, 1: 2, 2: 1}
                  for k, (g, func) in enumerate(GATES):
                      p = ps.tile([H, S, BL], f32, name=f"psum{g}", bufs=PBUFS[g])
                      nc.tensor.matmul(
                          p[:, ilo:, :], wi[:, g * H : (g + 1) * H], it[:, ilo:, :],
                          start=True, stop=False,
                      )
                      pt.append(p)
                      if k == 0:
                          nc.tensor.matmul(
                              p[:, sE:, :], wtiles[4 + g], hist[:, cp, sE:, :],
                              start=False, stop=False,
                          )
                          nc.tensor.matmul(
                              p[:, s0E:, :], wtiles[g], hist[:, cp, s0E - 1 : 31, :],
                              start=False, stop=True,
                          )

                  sig = [None] * 4
                  u = None
                  for k, (g, func) in enumerate(GATES):
                      p = pt[k]
                      if k > 0:
                          nc.tensor.matmul(
                              p[:, sE:, :], wtiles[4 + g], hist[:, cp, sE:, :],
                              start=False, stop=False,
                          )
                          nc.tensor.matmul(
                              p[:, s0E:, :], wtiles[g], hist[:, cp, s0E - 1 : 31, :],
                              start=False, stop=True,
                          )
                      sg = wpool.tile([H, S, BL], f32, name=f"sig{g}")
                      nc.scalar.activation(
                          sg[:, s_lo:, :], p[:, s_lo:, :], func,
                          bias=tbt[:, g : g + 1], scale=1.0,
                      )
                      sig[g] = sg
                      if g == 3:
                          # h_n = c_{n-1} * sigmoid(o); lands in hist column n
                          nc.vector.tensor_mul(
                              hist[:, n, s_lo:, :], c_prev[:, s_lo:, :], sg[:, s_lo:, :]
                          )
                          if n >= 31 and not skip_out:
                              r = n - 31
                              if r % 4 == 0:
                                  stg = gpool.tile([H, 4, BL, S], f32, name="stg")
                              nc.vector.tensor_copy(
                                  stg[:, r % 4], hist_r[:, :, r : r + 32, r]
                              )
                              if r % 4 == 3 and r < 28:
                                  nc.sync.dma_start(
                                      out=out_v[:, r - 3 : r + 1, :, :], in_=stg
                                  )
                              elif r >= 28:
                                  # last block: per-row DMAs so the tail drains
                                  # as rows complete instead of all after n=62
                                  nc.sync.dma_start(
                                      out=out_v[:, r : r + 1, :, :],
                                      in_=stg[:, r % 4 : r % 4 + 1],
                                  )
                      elif g == 1:
                          u = wpool.tile([H, S, BL], f32, name="u")
                          nc.gpsimd.tensor_mul(
                              u[:, s_lo:, :], sig[0][:, s_lo:, :], sig[1][:, s_lo:, :]
                          )

                  cn = spool.tile([H, S, BL], f32, name="c_st")
                  nc.vector.tensor_mul(cn[:, s_lo:, :], sig[2][:, s_lo:, :], c_prev[:, s_lo:, :])
                  nc.vector.tensor_add(cn[:, s_lo:, :], cn[:, s_lo:, :], u[:, s_lo:, :])
                  c_prev = cn

    _split_multi_waits(nc)
    return nc


STOS_DT = "f32r"

def _prep(image, itos_w, itos_b, stos_w, stos_b, h0, c0):
    image = np.ascontiguousarray(np.asarray(image, dtype=np.float32))
    itos_w = np.asarray(itos_w, dtype=np.float32)
    itos_b = np.asarray(itos_b, dtype=np.float32)
    stos_w = np.asarray(stos_w, dtype=np.float32)
    stos_b = np.asarray(stos_b, dtype=np.float32)
    h0 = np.asarray(h0, dtype=np.float32)
    c0 = np.asarray(c0, dtype=np.float32)

    import ml_dtypes
    sdt_np = np.float32 if STOS_DT == "f32r" else ml_dtypes.bfloat16
    wst = np.empty((8, H, H), sdt_np)
    for t in range(2):
        for g in range(4):
            wst[4 * t + g] = stos_w[g * H : (g + 1) * H, :, t].T
    import ml_dtypes
    wit = np.ascontiguousarray(itos_w.T.astype(ml_dtypes.bfloat16))
    tbd = np.ascontiguousarray((itos_b + stos_b).reshape(4, H).T)
    h0b = np.ascontiguousarray(np.broadcast_to(h0[:, :, None], (H, S, BL)).astype(sdt_np))
    c0b = np.ascontiguousarray(np.broadcast_to(c0[:, :, None], (H, S, BL)))

    shared = {"wst": wst, "wit": wit, "tbd": tbd, "h0b": h0b, "c0b": c0b}
    in_maps = []
    for core in range(NCORES):
        b0 = core * BL
        img_loc = image[b0 : b0 + BL].transpose(1, 0, 2, 3)  # (C, BL, S, S)
        # img_steps[c, n, s, b] = image[b, c, s, n-s]
        img_steps = np.zeros((C, NSEQ, S, BL), ml_dtypes.bfloat16)
        for s in range(S):
            img_steps[:, s : s + S, s, :] = img_loc[:, :, s, :].transpose(0, 2, 1)
        in_maps.append({"img": img_steps, **shared})
    return in_maps


def kernel(image, itos_w, itos_b, stos_w, stos_b, h0, c0):
    from concourse.bass_utils import run_bass_kernel_spmd

    if "nc" not in _cache:
        _cache["nc"] = _build()
    nc = _cache["nc"]
    in_maps = _prep(image, itos_w, itos_b, stos_w, stos_b, h0, c0)
    # retry guards against rare transient device glitches (non-finite output)
    for attempt in range(3):
        res = run_bass_kernel_spmd(nc, in_maps, core_ids=list(range(NCORES)))
        # per-core out is [S, H, BL, S] (s, h, b, j) -> (b, h, s, j)
        out = np.concatenate(
            [res.results[i]["out"].transpose(2, 1, 0, 3) for i in range(NCORES)],
            axis=0,
        )
        if np.isfinite(out).all():
            return out
    return out



# revision 2
# speedup vs baseline: 1.4950x; 1.4950x over previous
"""DiagonalLSTM Trainium2 kernel (v2).

Data-parallel over batch across 8 NeuronCores (16 samples/core). Two
structural changes over v1:

1. Host-side prologue: cells (s, n) with n < s ("pre-active" rows) evolve
   identically for every batch sample because h0/c0 are broadcast across the
   batch and the image contributes nothing there. Their evolution (the
   entry states h_ent(s) = h(s, s-1), c_ent(s) = c(s, s-1)) is computed once
   on the host and injected: h_ent via static diagonal slots in the h history
   tile, c_ent via prefilled c state tiles. The device scan then touches only
   the ACTIVE band rows [max(0, n-31), min(n, 31)] each step - 33% less work
   on every engine than the full 32-row band.

2. The h history tile (fp16) doubles as the output: column n is DMA'd to DRAM
   right after it is written, and the host unskews. No on-chip output
   staging at all.

Per step: 12 fp16 matmuls (4 itos + 2x4 stos taps, all over the same active
band, no padding), 4 activations (sigmoid o/i/f + tanh g, order o,i,g,f,
fp16 outputs), h = c_prev*sig(o) on DVE straight into the hist column,
u = sig(i)*tanh(g) on GpSimd, c updates (2 ops) on DVE with fp16 state in
double-buffered tiles (parity-alternating to keep the Pool reader off the
c-update WAR path). All DVE ops are all-fp16 for the 2x DVE rate.

Cost-model timeline: ~100us/core (PE 55us, Act 68us busy).
"""

import numpy as np

B, C, H, S = 128, 128, 128, 32
NCORES = 8
BL = B // NCORES
NSEQ = 2 * S - 1  # 63

_cache = {}


def _split_multi_waits(nc):
    # This walrus build accepts a single sync-wait per instruction; hoist
    # extras onto standalone EventSemaphore instructions on the same engine.
    import bass_rust
    import concourse.mybir as mybir

    n_split = 0
    for f in nc.m.functions:
        for bb in f.blocks:
            insts = bb.instructions
            new = []
            for ins in insts:
                si = ins.sync_info
                if si is not None and len(si.on_wait) > 1:
                    waits = list(si.on_wait)
                    for k, w in enumerate(waits[:-1]):
                        ev = mybir.InstEventSemaphore(
                            name=f"{ins.name}_splitw{k}",
                            engine=ins.engine,
                            ins=[],
                            outs=[],
                            sync_info=bass_rust.SyncInfo(on_wait=[w], on_update=[]),
                        )
                        new.append(ev)
                        n_split += 1
                    ins.sync_info = bass_rust.SyncInfo(
                        on_wait=[waits[-1]], on_update=list(si.on_update)
                    )
                new.append(ins)
            insts[:] = new
    return n_split


def _build(reps=1):
    import contextlib

    import concourse.bass as bass
    import concourse.mybir as mybir
    from concourse.tile import TileContext

    f32 = mybir.dt.float32
    f16 = mybir.dt.float16
    AF = mybir.ActivationFunctionType

    nc = bass.Bass()
    img = nc.dram_tensor("img", [C, NSEQ, S, BL], f16, kind="ExternalInput")
    wst = nc.dram_tensor("wst", [8, H, H], f16, kind="ExternalInput")
    wit = nc.dram_tensor("wit", [C, 4 * H], f16, kind="ExternalInput")
    tbd = nc.dram_tensor("tbd", [H, 4], f32, kind="ExternalInput")
    hseed = nc.dram_tensor("hseed", [H, S, BL], f16, kind="ExternalInput")
    cseed = nc.dram_tensor("cseed", [H, S, BL], f16, kind="ExternalInput")
    out = nc.dram_tensor("out", [H, NSEQ, S, BL], f16, kind="ExternalOutput")

    # gate order in psum/weights: o, i, g, f (o first: h-recurrence chain;
    # f last: the c-update chain has a full step of slack)
    GATES = ((3, AF.Sigmoid), (0, AF.Sigmoid), (1, AF.Tanh), (2, AF.Sigmoid))

    with TileContext(nc) as tc:
        with (
            tc.tile_pool(name="const", bufs=1) as cpool,
            tc.tile_pool(name="histp", bufs=1) as hpool,
            tc.tile_pool(name="state", bufs=1) as spool,
            tc.tile_pool(name="work", bufs=3) as wpool,
            tc.tile_pool(name="imgp", bufs=6) as ipool,
            tc.tile_pool(name="ps", bufs=2, space="PSUM") as ps,
        ):
            # weights: wst[tap*4+gate] -> lhsT [in 128, out 128], one DMA
            wstt = cpool.tile([H, 8, H], f16, name="wstt")
            nc.gpsimd.dma_start(out=wstt, in_=wst.rearrange("k i o -> i k o"))
            wtiles = [wstt[:, k, :] for k in range(8)]
            wi = cpool.tile([C, 4 * H], f16, name="wi")
            nc.gpsimd.dma_start(out=wi, in_=wit[:, :])
            tbt = cpool.tile([H, 4], f32, name="tbt")
            nc.gpsimd.dma_start(out=tbt, in_=tbd[:, :])

            # h history: [H, col n, row idx s+1, b]; idx 0 = zero row for the
            # shifted tap at the bottom boundary. Entry states h_ent(s) live
            # at (col s-1, idx s+1) (col 63 for s=0) - written ONCE here; the
            # per-step writes below never touch those slots (step n writes
            # col n, idx <= min(n,31)+1 < n+2), so reps reuse them.
            hist = hpool.tile([H, 64, 33, BL], f16, name="hist")
            nc.vector.memset(hist[:, :, 0, :], 0.0)
            nc.sync.dma_start(out=hist[:, 63, 1, :], in_=hseed[:, 0, :])
            for s in range(1, S):
                nc.sync.dma_start(out=hist[:, s - 1, s + 1, :], in_=hseed[:, s, :])

            c_st = [spool.tile([H, S, BL], f16, name=f"c{p}") for p in range(2)]

            loop_ctx = tc.For_i(0, reps, 1) if reps > 1 else contextlib.nullcontext()
            with loop_ctx:
                # c entry states: rows of both parities are consumed before
                # first write, and overwritten during the scan -> per rep.
                nc.sync.dma_start(out=c_st[0], in_=cseed[:, :, :])
                nc.sync.dma_start(out=c_st[1], in_=cseed[:, :, :])
                for n in range(NSEQ):
                    lo, hi = max(0, n - 31), min(n, 31)
                    nb = hi - lo + 1
                    cp = n - 1 if n > 0 else 63
                    c_prev, c_new = c_st[n % 2], c_st[1 - n % 2]

                    if n % 2 == 0:
                        n_hi = min(n + 2, NSEQ)
                        dhi = min(n_hi - 1, 31)
                        it2 = ipool.tile([C, 2, S, BL], f16, name="imgt")
                        nc.sync.dma_start(
                            out=it2[:, : n_hi - n, lo : dhi + 1, :],
                            in_=img[:, n:n_hi, lo : dhi + 1, :],
                        )
                    it = it2[:, n % 2]

                    # psum: per-gate bank, double-buffered; all taps cover the
                    # same [lo:hi+1] band.
                    pt = []
                    for k, (g, func) in enumerate(GATES):
                        p = ps.tile([H, S, BL], f32, name=f"psum{g}", bufs=2)
                        nc.tensor.matmul(
                            p[:, lo : hi + 1, :],
                            wi[:, g * H : (g + 1) * H],
                            it[:, lo : hi + 1, :],
                            start=True,
                            stop=False,
                        )
                        pt.append(p)

                    sig = {}
                    for k, (g, func) in enumerate(GATES):
                        p = pt[k]
                        nc.tensor.matmul(
                            p[:, lo : hi + 1, :],
                            wtiles[4 + g],
                            hist[:, cp, lo + 1 : hi + 2, :],
                            start=False,
                            stop=False,
                        )
                        nc.tensor.matmul(
                            p[:, lo : hi + 1, :],
                            wtiles[g],
                            hist[:, cp, lo : hi + 1, :],
                            start=False,
                            stop=True,
                        )
                        sg = wpool.tile([H, S, BL], f16, name=f"sig{g}")
                        nc.scalar.activation(
                            sg[:, lo : hi + 1, :],
                            p[:, lo : hi + 1, :],
                            func,
                            bias=tbt[:, g : g + 1],
                            scale=1.0,
                        )
                        sig[g] = sg
                        if g == 3:
                            # h_n = c_{n-1} * sig(o) -> hist col n, then out
                            nc.vector.tensor_mul(
                                hist[:, n, lo + 1 : hi + 2, :],
                                c_prev[:, lo : hi + 1, :],
                                sg[:, lo : hi + 1, :],
                            )
                            nc.sync.dma_start(
                                out=out[:, n, lo : hi + 1, :],
                                in_=hist[:, n, lo + 1 : hi + 2, :],
                            )
                        elif g == 1:
                            u = wpool.tile([H, S, BL], f16, name="u")
                            nc.gpsimd.tensor_mul(
                                u[:, lo : hi + 1, :],
                                sig[0][:, lo : hi + 1, :],
                                sig[1][:, lo : hi + 1, :],
                            )

                    m = wpool.tile([H, S, BL], f16, name="m")
                    nc.vector.tensor_mul(
                        m[:, lo : hi + 1, :],
                        sig[2][:, lo : hi + 1, :],
                        c_prev[:, lo : hi + 1, :],
                    )
                    nc.vector.tensor_add(
                        c_new[:, lo : hi + 1, :],
                        m[:, lo : hi + 1, :],
                        u[:, lo : hi + 1, :],
                    )

    _split_multi_waits(nc)
    return nc


def _prologue(stos_w, itos_b, stos_b, h0, c0):
    """Evolve the batch-independent pre-active rows on the host.

    Returns entry states h_ent, c_ent [H, S]: the (h, c) of row s just before
    its first active step n = s."""

    def sig(x):
        return 1.0 / (1.0 + np.exp(-x))

    b = itos_b + stos_b
    hp = np.broadcast_to(h0, (H, S)).copy()
    cp = np.broadcast_to(c0, (H, S)).copy()
    h_ent = np.zeros((H, S), np.float32)
    c_ent = np.zeros((H, S), np.float32)
    h_ent[:, 0] = h0[:, 0]
    c_ent[:, 0] = c0[:, 0]
    for n in range(0, S - 1):
        rows = np.arange(n + 1, S)
        sh = hp[:, rows - 1]
        g = stos_w[:, :, 0] @ sh + stos_w[:, :, 1] @ hp[:, rows] + b[:, None]
        i_, g_, f_, o_ = np.split(g, 4, axis=0)
        c_n = sig(f_) * cp[:, rows] + sig(i_) * np.tanh(g_)
        h_n = cp[:, rows] * sig(o_)
        hp[:, rows] = h_n
        cp[:, rows] = c_n
        h_ent[:, n + 1] = h_n[:, 0]
        c_ent[:, n + 1] = c_n[:, 0]
    return h_ent, c_ent


def _prep(image, itos_w, itos_b, stos_w, stos_b, h0, c0):
    image = np.ascontiguousarray(np.asarray(image, dtype=np.float32))
    itos_w = np.asarray(itos_w, dtype=np.float32)
    itos_b = np.asarray(itos_b, dtype=np.float32)
    stos_w = np.asarray(stos_w, dtype=np.float32)
    stos_b = np.asarray(stos_b, dtype=np.float32)
    h0 = np.asarray(h0, dtype=np.float32)
    c0 = np.asarray(c0, dtype=np.float32)

    f16 = np.float16
    h_ent, c_ent = _prologue(stos_w, itos_b, stos_b, h0, c0)
    hseed = np.ascontiguousarray(
        np.broadcast_to(h_ent[:, :, None], (H, S, BL)).astype(f16)
    )
    cseed = np.ascontiguousarray(
        np.broadcast_to(c_ent[:, :, None], (H, S, BL)).astype(f16)
    )

    wst = np.empty((8, H, H), f16)
    for t in range(2):
        for g in range(4):
            wst[4 * t + g] = stos_w[g * H : (g + 1) * H, :, t].T
    wit = np.ascontiguousarray(itos_w.T.astype(f16))
    tbd = np.ascontiguousarray((itos_b + stos_b).reshape(4, H).T)

    shared = {"wst": wst, "wit": wit, "tbd": tbd, "hseed": hseed, "cseed": cseed}
    in_maps = []
    for core in range(NCORES):
        b0 = core * BL
        img_loc = image[b0 : b0 + BL].transpose(1, 0, 2, 3)  # (C, BL, S, S)
        # img_steps[c, n, s, b] = image[b, c, s, n-s]
        img_steps = np.zeros((C, NSEQ, S, BL), f16)
        for s in range(S):
            img_steps[:, s : s + S, s, :] = img_loc[:, :, s, :].transpose(0, 2, 1)
        in_maps.append({"img": img_steps, **shared})
    return in_maps


def _unskew_host(res):
    # per-core out is [H, NSEQ, S, BL] fp16: col n, row s, sample b with
    # h(s, n) = out[b, :, s, n-s]
    full = np.empty((B, H, S, S), np.float32)
    for core in range(NCORES):
        A = np.asarray(res[core]["out"], dtype=np.float32)  # (H, 63, 32, 16)
        b0 = core * BL
        for s in range(S):
            # (H, 32j, 16b) -> (16b, H, 32j)
            full[b0 : b0 + BL, :, s, :] = A[:, s : s + S, s, :].transpose(2, 0, 1)
    return full


def kernel(image, itos_w, itos_b, stos_w, stos_b, h0, c0):
    from concourse.bass_utils import run_bass_kernel_spmd

    if "nc" not in _cache:
        _cache["nc"] = _build()
    nc = _cache["nc"]
    in_maps = _prep(image, itos_w, itos_b, stos_w, stos_b, h0, c0)
    # retry guards against rare transient device glitches (non-finite output)
    for attempt in range(3):
        res = run_bass_kernel_spmd(nc, in_maps, core_ids=list(range(NCORES)))
        out = _unskew_host(res.results)
        if np.isfinite(out).all():
            return out
    return out


# revision 12
# speedup vs baseline: 2.4109x; 1.6126x over previous
"""DiagonalLSTM Trainium2 kernel (v2.1).

Data-parallel over batch across 8 NeuronCores (16 samples/core). Two
structural changes over v1:

1. Host-side prologue: cells (s, n) with n < s ("pre-active" rows) evolve
   identically for every batch sample because h0/c0 are broadcast across the
   batch and the image contributes nothing there. Their evolution (the
   entry states h_ent(s) = h(s, s-1), c_ent(s) = c(s, s-1)) is computed once
   on the host and injected: h_ent via static diagonal slots in the h history
   tile, c_ent via prefilled c state tiles. The device scan then touches only
   the ACTIVE band rows [max(0, n-31), min(n, 31)] each step - 33% less work
   on every engine than the full 32-row band.

2. The h history tile (fp16) doubles as the output: column n is DMA'd to DRAM
   right after it is written, and the host unskews. No on-chip output
   staging at all.

Per step: 12 fp16 matmuls (4 itos + 2x4 stos taps, all over the same active
band, no padding), 4 activations (sigmoid o/i/f + tanh g, order o,i,g,f,
fp16 outputs), h = c_prev*sig(o) on DVE straight into the hist column,
u = sig(i)*tanh(g), c updates (2 ops) on DVE with fp16 state in
double-buffered tiles (parity-alternating to keep readers off the
c-update WAR path). All DVE ops are all-fp16 for the 2x DVE rate.

Cost-model timeline: ~132us/core; measured 119.7us on HW (reps-loop dispatch-delta method).
"""

import numpy as np

B, C, H, S = 128, 128, 128, 32
NCORES = 8
BL = B // NCORES
NSEQ = 2 * S - 1  # 63

_cache = {}


def _split_multi_waits(nc):
    # This walrus build accepts a single sync-wait per instruction; hoist
    # extras onto standalone EventSemaphore instructions on the same engine.
    import bass_rust
    import concourse.mybir as mybir

    n_split = 0
    for f in nc.m.functions:
        for bb in f.blocks:
            insts = bb.instructions
            new = []
            for ins in insts:
                si = ins.sync_info
                if si is not None and len(si.on_wait) > 1:
                    waits = list(si.on_wait)
                    for k, w in enumerate(waits[:-1]):
                        ev = mybir.InstEventSemaphore(
                            name=f"{ins.name}_splitw{k}",
                            engine=ins.engine,
                            ins=[],
                            outs=[],
                            sync_info=bass_rust.SyncInfo(on_wait=[w], on_update=[]),
                        )
                        new.append(ev)
                        n_split += 1
                    ins.sync_info = bass_rust.SyncInfo(
                        on_wait=[waits[-1]], on_update=list(si.on_update)
                    )
                new.append(ins)
            insts[:] = new
    return n_split


def _build(reps=1):
    import contextlib

    import concourse.bass as bass
    import concourse.mybir as mybir
    from concourse.tile import TileContext

    f32 = mybir.dt.float32
    f16 = mybir.dt.float16
    AF = mybir.ActivationFunctionType

    nc = bass.Bass()
    img = nc.dram_tensor("img", [C, NSEQ, S, BL], f16, kind="ExternalInput")
    wst = nc.dram_tensor("wst", [8, H, H], f16, kind="ExternalInput")
    wit = nc.dram_tensor("wit", [C, 4 * H], f16, kind="ExternalInput")
    tbd = nc.dram_tensor("tbd", [H, 4], f32, kind="ExternalInput")
    hseed = nc.dram_tensor("hseed", [H, S, BL], f16, kind="ExternalInput")
    cseed = nc.dram_tensor("cseed", [H, S, BL], f16, kind="ExternalInput")
    out = nc.dram_tensor("out", [H, NSEQ, S, BL], f16, kind="ExternalOutput")

    # gate order in psum/weights: o, i, g, f (o first: h-recurrence chain;
    # f last: the c-update chain has a full step of slack)
    GATES = ((3, AF.Sigmoid), (0, AF.Sigmoid), (1, AF.Tanh), (2, AF.Sigmoid))

    with TileContext(nc) as tc:
        with (
            tc.tile_pool(name="const", bufs=1) as cpool,
            tc.tile_pool(name="histp", bufs=1) as hpool,
            tc.tile_pool(name="state", bufs=1) as spool,
            tc.tile_pool(name="work", bufs=3) as wpool,
            tc.tile_pool(name="imgp", bufs=6) as ipool,
            tc.tile_pool(name="ps", bufs=2, space="PSUM") as ps,
        ):
            # weights: wst[tap*4+gate] -> lhsT [in 128, out 128], one DMA
            wstt = cpool.tile([H, 8, H], f16, name="wstt")
            nc.gpsimd.dma_start(out=wstt, in_=wst.rearrange("k i o -> i k o"))
            wtiles = [wstt[:, k, :] for k in range(8)]
            wi = cpool.tile([C, 4 * H], f16, name="wi")
            nc.gpsimd.dma_start(out=wi, in_=wit[:, :])
            tbt = cpool.tile([H, 4], f32, name="tbt")
            nc.gpsimd.dma_start(out=tbt, in_=tbd[:, :])
            # static c entry seeds; copied (not DMA'd) into c_st each rep so
            # the rep-boundary chain avoids DMA-generation latency
            cs = [cpool.tile([H, S, BL], f16, name=f"cs{p}") for p in range(2)]
            nc.sync.dma_start(out=cs[0], in_=cseed[:, :, :])
            nc.sync.dma_start(out=cs[1], in_=cseed[:, :, :])

            # h history: [H, col n, row idx s+1, b]; idx 0 = zero row for the
            # shifted tap at the bottom boundary. Entry states h_ent(s) live
            # at (col s-1, idx s+1) (col 63 for s=0) - written ONCE here; the
            # per-step writes below never touch those slots (step n writes
            # col n, idx <= min(n,31)+1 < n+2), so reps reuse them.
            hist = hpool.tile([H, 64, 33, BL], f16, name="hist")
            nc.vector.memset(hist[:, :, 0, :], 0.0)
            nc.sync.dma_start(out=hist[:, 63, 1, :], in_=hseed[:, 0, :])
            for s in range(1, S):
                nc.sync.dma_start(out=hist[:, s - 1, s + 1, :], in_=hseed[:, s, :])

            c_st = [spool.tile([H, S, BL], f16, name=f"c{p}") for p in range(2)]

            loop_ctx = tc.For_i(0, reps, 1) if reps > 1 else contextlib.nullcontext()
            with loop_ctx:
                # c entry states: rows of both parities are consumed before
                # first write, and overwritten during the scan -> per rep.
                nc.vector.tensor_copy(c_st[0], cs[0])
                nc.vector.tensor_copy(c_st[1], cs[1])
                for n in range(NSEQ):
                    lo, hi = max(0, n - 31), min(n, 31)
                    cp = n - 1 if n > 0 else 63
                    c_prev, c_new = c_st[n % 2], c_st[1 - n % 2]

                    if n % 2 == 0:
                        n_hi = min(n + 2, NSEQ)
                        dhi = min(n_hi - 1, 31)
                        it2 = ipool.tile([C, 2, S, BL], f16, name="imgt")
                        nc.sync.dma_start(
                            out=it2[:, : n_hi - n, lo : dhi + 1, :],
                            in_=img[:, n:n_hi, lo : dhi + 1, :],
                        )
                    it = it2[:, n % 2]

                    # psum: per-gate bank, double-buffered; all taps cover the
                    # same [lo:hi+1] band.
                    pt = []
                    for k, (g, func) in enumerate(GATES):
                        p = ps.tile([H, S, BL], f32, name=f"psum{g}", bufs=2)
                        nc.tensor.matmul(
                            p[:, lo : hi + 1, :],
                            wi[:, g * H : (g + 1) * H],
                            it[:, lo : hi + 1, :],
                            start=True,
                            stop=False,
                        )
                        pt.append(p)

                    sig = {}
                    for k, (g, func) in enumerate(GATES):
                        p = pt[k]
                        nc.tensor.matmul(
                            p[:, lo : hi + 1, :],
                            wtiles[4 + g],
                            hist[:, cp, lo + 1 : hi + 2, :],
                            start=False,
                            stop=False,
                        )
                        nc.tensor.matmul(
                            p[:, lo : hi + 1, :],
                            wtiles[g],
                            hist[:, cp, lo : hi + 1, :],
                            start=False,
                            stop=True,
                        )
                        sg = wpool.tile([H, S, BL], f16, name=f"sig{g}")
                        nc.scalar.activation(
                            sg[:, lo : hi + 1, :],
                            p[:, lo : hi + 1, :],
                            func,
                            bias=tbt[:, g : g + 1],
                            scale=1.0,
                        )
                        sig[g] = sg
                        if g == 3:
                            # h_n = c_{n-1} * sig(o) -> hist col n, then out
                            nc.vector.tensor_mul(
                                hist[:, n, lo + 1 : hi + 2, :],
                                c_prev[:, lo : hi + 1, :],
                                sg[:, lo : hi + 1, :],
                            )
                            nc.sync.dma_start(
                                out=out[:, n, lo : hi + 1, :],
                                in_=hist[:, n, lo + 1 : hi + 2, :],
                            )
                        elif g == 1:
                            u = wpool.tile([H, S, BL], f16, name="u")
                            nc.vector.tensor_mul(
                                u[:, lo : hi + 1, :],
                                sig[0][:, lo : hi + 1, :],
                                sig[1][:, lo : hi + 1, :],
                            )

                    m = wpool.tile([H, S, BL], f16, name="m")
                    nc.vector.tensor_mul(
                        m[:, lo : hi + 1, :],
                        sig[2][:, lo : hi + 1, :],
                        c_prev[:, lo : hi + 1, :],
                    )
                    nc.vector.tensor_add(
                        c_new[:, lo : hi + 1, :],
                        m[:, lo : hi + 1, :],
                        u[:, lo : hi + 1, :],
                    )

    _split_multi_waits(nc)
    return nc


def _prologue(stos_w, itos_b, stos_b, h0, c0):
    """Evolve the batch-independent pre-active rows on the host.

    Returns entry states h_ent, c_ent [H, S]: the (h, c) of row s just before
    its first active step n = s."""

    def sig(x):
        return 1.0 / (1.0 + np.exp(-x))

    b = itos_b + stos_b
    hp = np.broadcast_to(h0, (H, S)).copy()
    cp = np.broadcast_to(c0, (H, S)).copy()
    h_ent = np.zeros((H, S), np.float32)
    c_ent = np.zeros((H, S), np.float32)
    h_ent[:, 0] = h0[:, 0]
    c_ent[:, 0] = c0[:, 0]
    for n in range(0, S - 1):
        rows = np.arange(n + 1, S)
        sh = hp[:, rows - 1]
        g = stos_w[:, :, 0] @ sh + stos_w[:, :, 1] @ hp[:, rows] + b[:, None]
        i_, g_, f_, o_ = np.split(g, 4, axis=0)
        c_n = sig(f_) * cp[:, rows] + sig(i_) * np.tanh(g_)
        h_n = cp[:, rows] * sig(o_)
        hp[:, rows] = h_n
        cp[:, rows] = c_n
        h_ent[:, n + 1] = h_n[:, 0]
        c_ent[:, n + 1] = c_n[:, 0]
    return h_ent, c_ent


def _prep(image, itos_w, itos_b, stos_w, stos_b, h0, c0):
    image = np.ascontiguousarray(np.asarray(image, dtype=np.float32))
    itos_w = np.asarray(itos_w, dtype=np.float32)
    itos_b = np.asarray(itos_b, dtype=np.float32)
    stos_w = np.asarray(stos_w, dtype=np.float32)
    stos_b = np.asarray(stos_b, dtype=np.float32)
    h0 = np.asarray(h0, dtype=np.float32)
    c0 = np.asarray(c0, dtype=np.float32)

    f16 = np.float16
    h_ent, c_ent = _prologue(stos_w, itos_b, stos_b, h0, c0)
    hseed = np.ascontiguousarray(
        np.broadcast_to(h_ent[:, :, None], (H, S, BL)).astype(f16)
    )
    cseed = np.ascontiguousarray(
        np.broadcast_to(c_ent[:, :, None], (H, S, BL)).astype(f16)
    )

    wst = np.empty((8, H, H), f16)
    for t in range(2):
        for g in range(4):
            wst[4 * t + g] = stos_w[g * H : (g + 1) * H, :, t].T
    wit = np.ascontiguousarray(itos_w.T.astype(f16))
    tbd = np.ascontiguousarray((itos_b + stos_b).reshape(4, H).T)

    shared = {"wst": wst, "wit": wit, "tbd": tbd, "hseed": hseed, "cseed": cseed}
    in_maps = []
    for core in range(NCORES):
        b0 = core * BL
        img_loc = image[b0 : b0 + BL].transpose(1, 0, 2, 3)  # (C, BL, S, S)
        # img_steps[c, n, s, b] = image[b, c, s, n-s]
        img_steps = np.zeros((C, NSEQ, S, BL), f16)
        for s in range(S):
            img_steps[:, s : s + S, s, :] = img_loc[:, :, s, :].transpose(0, 2, 1)
        in_maps.append({"img": img_steps, **shared})
    return in_maps


def _unskew_host(res):
    # per-core out is [H, NSEQ, S, BL] fp16: col n, row s, sample b with
    # h(s, n) = out[b, :, s, n-s]
    full = np.empty((B, H, S, S), np.float32)
    for core in range(NCORES):
        A = np.asarray(res[core]["out"], dtype=np.float32)  # (H, 63, 32, 16)
        b0 = core * BL
        for s in range(S):
            # (H, 32j, 16b) -> (16b, H, 32j)
            full[b0 : b0 + BL, :, s, :] = A[:, s : s + S, s, :].transpose(2, 0, 1)
    return full


def kernel(image, itos_w, itos_b, stos_w, stos_b, h0, c0):
    from concourse.bass_utils import run_bass_kernel_spmd

    if "nc" not in _cache:
        _cache["nc"] = _build()
    nc = _cache["nc"]
    in_maps = _prep(image, itos_w, itos_b, stos_w, stos_b, h0, c0)
    # retry guards against rare transient device glitches (non-finite output)
    for attempt in range(3):
        res = run_bass_kernel_spmd(nc, in_maps, core_ids=list(range(NCORES)))
        out = _unskew_host(res.results)
        if np.isfinite(out).all():
            return out
    return out
